# revision 15
# baseline (speedup 1.0000x reference)
"""ETNN messager layer on 8 Trainium2 NeuronCores — v5 (streaming, no indirect DMA).

Receiver-sharded: core k owns receivers [k*12500,(k+1)*12500). The host
stages per-edge data as contiguous streams (pure data movement: gather /
transpose / one-hot layout); the device does all model FLOPs and sees only
contiguous DMA:

- xsT / xrT: per-edge x_send / x_rec rows, transposed to [H, slots] bf16,
  consumed directly as matmul lhsT tiles (z = xs@Wa + xr@Wb + ea@Wc + b1,
  BN folded into W1 on host).
- Edges are grouped into fixed-size receiver blocks: each block = up to 128
  consecutive receivers whose edges fit in 512 slots (zero-padded). The
  segment-sum is a one-hot matmul per 128-slot subtile: out_block[128,H]
  += Sel^T @ ff accumulated in PSUM (fp32), then written contiguously to
  the output table. Sel columns of pad slots are all-zero, so pads vanish.
- msg = z*sigmoid(z); gate = sigmoid(msg.W2+b2); ff = msg*gate.

The block structure is data-dependent but the program is uniform (every
block = 4 subtiles), so one SPMD program serves all 8 cores; the host
compacts each core's [nblocks*128, H] output using its block bases.
"""

import numpy as np
from ml_dtypes import bfloat16

import concourse.tile as tile
from concourse import bacc, bass, mybir
from concourse.bass_utils import run_bass_kernel_spmd

N = 100000
E = 500000
H = 128
INV = 16
NCORES = 8
NLOC = N // NCORES           # 12500 receivers per core
BSLOT = 512                  # slots per receiver block (4 subtiles)
NSUB = BSLOT // 128
BN_EPS = 1e-5

_prog_cache = {}


def _build(b2val: float, nblocks: int):
    key = (round(b2val, 9), nblocks)
    if key in _prog_cache:
        return _prog_cache[key]
    slots = nblocks * BSLOT

    nc = bacc.Bacc("TRN2", target_bir_lowering=False, debug=False)
    dt = mybir.dt
    xst = nc.dram_tensor("xst", [128, slots], dt.bfloat16, kind="ExternalInput")
    xrt = nc.dram_tensor("xrt", [128, slots], dt.bfloat16, kind="ExternalInput")
    sel = nc.dram_tensor("sel", [128, slots], dt.bfloat16, kind="ExternalInput")
    eat = nc.dram_tensor("eat", [INV + 1, slots], dt.bfloat16,
                         kind="ExternalInput")
    wa = nc.dram_tensor("wa", [H, H], dt.bfloat16, kind="ExternalInput")
    wb = nc.dram_tensor("wb", [H, H], dt.bfloat16, kind="ExternalInput")
    wc = nc.dram_tensor("wc", [INV + 1, H], dt.bfloat16, kind="ExternalInput")
    w2w = nc.dram_tensor("w2w", [128, NSUB, H], dt.bfloat16,
                         kind="ExternalInput")
    outp = nc.dram_tensor("outp", [nblocks * 128, H], dt.float16,
                          kind="ExternalOutput")

    with tile.TileContext(nc) as tc:
        with tc.tile_pool(name="const", bufs=1) as cp, \
             tc.tile_pool(name="strm", bufs=3) as stp, \
             tc.tile_pool(name="ea", bufs=3) as eap, \
             tc.tile_pool(name="sgp", bufs=3) as sgp, \
             tc.tile_pool(name="msg", bufs=3) as mp, \
             tc.tile_pool(name="ff", bufs=3) as fp_, \
             tc.tile_pool(name="small", bufs=4) as sp, \
             tc.tile_pool(name="oc", bufs=4) as ocp, \
             tc.tile_pool(name="pmp", bufs=3, space="PSUM") as pmp, \
             tc.tile_pool(name="obp", bufs=4, space="PSUM") as obp:
            wa_sb = cp.tile([H, H], dt.bfloat16)
            wb_sb = cp.tile([H, H], dt.bfloat16)
            wc_sb = cp.tile([INV + 1, H], dt.bfloat16)
            w2w_sb = cp.tile([128, NSUB, H], dt.bfloat16)
            b2t = cp.tile([128, 1], dt.float32)
            nc.vector.memset(b2t[:], b2val)
            nc.sync.dma_start(out=wa_sb[:], in_=wa[:, :])
            nc.sync.dma_start(out=wb_sb[:], in_=wb[:, :])
            nc.sync.dma_start(out=wc_sb[:], in_=wc[:, :])
            nc.sync.dma_start(out=w2w_sb[:], in_=w2w[:, :, :])

            for t in range(nblocks):
                ss = slice(t * BSLOT, (t + 1) * BSLOT)
                xs_sb = stp.tile([128, BSLOT], dt.bfloat16, tag="xs")
                xr_sb = stp.tile([128, BSLOT], dt.bfloat16, tag="xr")
                se_sb = stp.tile([128, BSLOT], dt.bfloat16, tag="se")
                nc.sync.dma_start(out=xs_sb[:], in_=xst[:, ss])
                nc.sync.dma_start(out=xr_sb[:], in_=xrt[:, ss])
                nc.sync.dma_start(out=se_sb[:], in_=sel[:, ss])
                ea_sb = eap.tile([INV + 1, BSLOT], dt.bfloat16, tag="ea")
                nc.sync.dma_start(out=ea_sb[:], in_=eat[:, ss])

                pm = pmp.tile([128, NSUB, 128], dt.float32, tag="pm")
                for j in range(NSUB):
                    js = slice(j * 128, (j + 1) * 128)
                    nc.tensor.matmul(out=pm[:, j, :], lhsT=xs_sb[:, js],
                                     rhs=wa_sb[:], start=True, stop=False)
                    nc.tensor.matmul(out=pm[:, j, :], lhsT=xr_sb[:, js],
                                     rhs=wb_sb[:], start=False, stop=False)
                    nc.tensor.matmul(out=pm[:, j, :], lhsT=ea_sb[:, js],
                                     rhs=wc_sb[:], start=False, stop=True)
                sg_t = sgp.tile([128, NSUB, 128], dt.bfloat16, tag="sg")
                nc.scalar.activation(
                    out=sg_t[:], in_=pm[:, :, :],
                    func=mybir.ActivationFunctionType.Sigmoid)
                msg = mp.tile([128, NSUB, 128], dt.bfloat16, tag="msg")
                nc.vector.tensor_tensor(out=msg[:], in0=pm[:, :, :],
                                        in1=sg_t[:], op=mybir.AluOpType.mult)
                tt = mp.tile([128, NSUB, 128], dt.bfloat16, tag="tt")
                nc.vector.tensor_tensor(out=tt[:], in0=msg[:], in1=w2w_sb[:],
                                        op=mybir.AluOpType.mult)
                red = sp.tile([128, NSUB], dt.float32, tag="red")
                nc.vector.tensor_reduce(out=red[:], in_=tt[:],
                                        axis=mybir.AxisListType.X,
                                        op=mybir.AluOpType.add)
                gate = sp.tile([128, NSUB], dt.bfloat16, tag="gate")
                nc.scalar.activation(
                    out=gate[:], in_=red[:],
                    func=mybir.ActivationFunctionType.Sigmoid,
                    bias=b2t[:, 0:1])
                ffl = fp_.tile([128, NSUB, 128], dt.bfloat16, tag="ff")
                nc.vector.tensor_tensor(
                    out=ffl[:], in0=msg[:],
                    in1=gate[:].to_broadcast([128, NSUB, 128]),
                    op=mybir.AluOpType.mult)
                ob = obp.tile([128, 128], dt.float32, tag="ob")
                for j in range(NSUB):
                    js = slice(j * 128, (j + 1) * 128)
                    nc.tensor.matmul(out=ob[:], lhsT=se_sb[:, js],
                                     rhs=ffl[:, j, :], start=(j == 0),
                                     stop=(j == NSUB - 1))
                oc = ocp.tile([128, 128], dt.float16, tag="oc")
                nc.vector.tensor_copy(out=oc[:], in_=ob[:])
                nc.sync.dma_start(out=outp[t * 128 : (t + 1) * 128, :],
                                  in_=oc[:])
    nc.compile()
    _prog_cache[key] = nc
    return nc


def _host_prep(x_send, x_rec, index, edge_attr, bn_gamma, bn_beta, bn_mean,
               bn_var, W1, b1, W2, b2):
    s = np.asarray(index[0], dtype=np.int64)
    r = np.asarray(index[1], dtype=np.int64)
    ea = np.asarray(edge_attr, dtype=np.float32)

    scale = np.asarray(bn_gamma) / np.sqrt(np.asarray(bn_var) + BN_EPS)
    shift = np.asarray(bn_beta) - np.asarray(bn_mean) * scale
    W1f = (np.asarray(W1) * scale[:, None]).astype(np.float32)
    b1f = (np.asarray(b1) + shift @ np.asarray(W1)).astype(np.float32)

    wa = W1f[:H].astype(bfloat16)
    wb = W1f[H : 2 * H].astype(bfloat16)
    wc = np.concatenate([W1f[2 * H :], b1f[None, :]], axis=0).astype(bfloat16)
    w2w = np.broadcast_to(
        np.asarray(W2, dtype=np.float32).reshape(1, 1, H), (128, NSUB, H)
    ).astype(bfloat16).copy()
    b2val = float(np.asarray(b2).reshape(-1)[0])

    xs_b = np.asarray(x_send, dtype=np.float32).astype(bfloat16)
    xr_b = np.asarray(x_rec, dtype=np.float32).astype(bfloat16)
    core_of = (r // NLOC).astype(np.int64)

    # per-core receiver-block partition: <=128 consecutive receivers whose
    # edge count fits in BSLOT slots
    percore = []
    nblocks = 0
    for k in range(NCORES):
        m = core_of == k
        sk, rk, eak = s[m], (r[m] - k * NLOC), ea[m]
        o = np.argsort(rk, kind="stable")
        sk, rk, eak = sk[o], rk[o], eak[o]
        deg = np.bincount(rk, minlength=NLOC)
        cum = np.concatenate([[0], np.cumsum(deg)])
        bases = [0]
        while bases[-1] < NLOC:
            base = bases[-1]
            n = int(np.searchsorted(cum, cum[base] + BSLOT, side="right")) - 1
            n = min(n - base, 128)
            assert n >= 1, "receiver degree exceeds block capacity"
            bases.append(base + n)
        percore.append((sk, rk, eak, cum, np.asarray(bases)))
        nblocks = max(nblocks, len(bases) - 1)

    slots = nblocks * BSLOT
    in_maps = []
    unpack_meta = []
    for k in range(NCORES):
        sk, rk, eak, cum, bases = percore[k]
        blk_marks = np.zeros(NLOC + 1, dtype=np.int64)
        blk_marks[bases[:-1]] = 1
        blk_of_r = np.cumsum(blk_marks[: NLOC]) - 1
        t_e = blk_of_r[rk]                       # block of each edge
        slot = t_e * BSLOT + (np.arange(rk.size) - cum[bases[t_e]])
        xst = np.zeros((128, slots), dtype=bfloat16)
        xrts = np.zeros((128, slots), dtype=bfloat16)
        selm = np.zeros((128, slots), dtype=bfloat16)
        eat = np.zeros((INV + 1, slots), dtype=np.float32)
        xst[:, slot] = xs_b[sk].T
        xrts[:, slot] = xr_b[k * NLOC + rk].T
        eat[:INV, slot] = eak.T
        eat[INV, slot] = 1.0
        # Sel lhsT tile per subtile: [e-part, r_rel free]; edge at slot has
        # partition slot%128 and free column r_rel inside its subtile group
        r_rel = (rk - bases[t_e]).astype(np.int64)
        p = slot % 128
        selm[p, (slot // 128) * 128 + r_rel] = 1.0
        in_maps.append({
            "xst": xst, "xrt": xrts, "sel": selm,
            "eat": eat.astype(bfloat16),
            "wa": wa, "wb": wb, "wc": wc, "w2w": w2w,
        })
        unpack_meta.append(bases)
    return in_maps, b2val, nblocks, unpack_meta


def kernel(**inputs) -> np.ndarray:
    in_maps, b2val, nblocks, unpack_meta = _host_prep(**inputs)
    nc = _build(b2val, nblocks)
    res = run_bass_kernel_spmd(nc, in_maps, core_ids=list(range(NCORES)))
    out = np.zeros((N, H), dtype=np.float32)
    for k in range(NCORES):
        op = np.asarray(res.results[k]["outp"], dtype=np.float32)
        bases = unpack_meta[k]
        for t in range(len(bases) - 1):
            nrec = bases[t + 1] - bases[t]
            out[k * NLOC + bases[t] : k * NLOC + bases[t + 1]] = \
                op[t * 128 : t * 128 + nrec]
    return out


# revision 17
# speedup vs baseline: 52620.3818x; 52620.3818x over previous
"""ETNN messager layer on 8 Trainium2 NeuronCores — v5 (streaming, no indirect DMA).

Receiver-sharded: core k owns receivers [k*12500,(k+1)*12500). The host
stages per-edge data as contiguous streams (pure data movement: gather /
transpose / one-hot layout); the device does all model FLOPs and sees only
contiguous DMA:

- xsT / xrT: per-edge x_send / x_rec rows, transposed to [H, slots] bf16,
  consumed directly as matmul lhsT tiles (z = xs@Wa + xr@Wb + ea@Wc + b1,
  BN folded into W1 on host).
- Edges are grouped into fixed-size receiver blocks: each block = up to 128
  consecutive receivers whose edges fit in 512 slots (zero-padded). The
  segment-sum is a one-hot matmul per 128-slot subtile: out_block[128,H]
  += Sel^T @ ff accumulated in PSUM (fp32), then written contiguously to
  the output table. Sel columns of pad slots are all-zero, so pads vanish.
- msg = z*sigmoid(z); gate = sigmoid(msg.W2+b2); ff = msg*gate.

The block structure is data-dependent but the program is uniform (every
block = 4 subtiles), so one SPMD program serves all 8 cores; the host
compacts each core's [nblocks*128, H] output using its block bases.
"""

import numpy as np
from ml_dtypes import bfloat16

import concourse.tile as tile
from concourse import bacc, bass, mybir
from concourse.bass_utils import run_bass_kernel_spmd

N = 100000
E = 500000
H = 128
INV = 16
NCORES = 8
NLOC = N // NCORES           # 12500 receivers per core
BSLOT = 512                  # slots per receiver block (4 subtiles)
NSUB = BSLOT // 128
BN_EPS = 1e-5

_prog_cache = {}


def _build(b2val: float, nblocks: int):
    key = (round(b2val, 9), nblocks)
    if key in _prog_cache:
        return _prog_cache[key]
    slots = nblocks * BSLOT

    nc = bacc.Bacc("TRN2", target_bir_lowering=False, debug=False)
    dt = mybir.dt
    xst = nc.dram_tensor("xst", [128, slots], dt.bfloat16, kind="ExternalInput")
    xrt = nc.dram_tensor("xrt", [128, slots], dt.bfloat16, kind="ExternalInput")
    sel = nc.dram_tensor("sel", [128, slots], dt.bfloat16, kind="ExternalInput")
    eat = nc.dram_tensor("eat", [INV + 1, slots], dt.bfloat16,
                         kind="ExternalInput")
    wa = nc.dram_tensor("wa", [H, H], dt.bfloat16, kind="ExternalInput")
    wb = nc.dram_tensor("wb", [H, H], dt.bfloat16, kind="ExternalInput")
    wc = nc.dram_tensor("wc", [INV + 1, H], dt.bfloat16, kind="ExternalInput")
    w2w = nc.dram_tensor("w2w", [128, NSUB, H], dt.bfloat16,
                         kind="ExternalInput")
    outp = nc.dram_tensor("outp", [nblocks * 128, H], dt.float16,
                          kind="ExternalOutput")

    with tile.TileContext(nc) as tc:
        with tc.tile_pool(name="const", bufs=1) as cp, \
             tc.tile_pool(name="strm", bufs=3) as stp, \
             tc.tile_pool(name="ea", bufs=3) as eap, \
             tc.tile_pool(name="sgp", bufs=3) as sgp, \
             tc.tile_pool(name="msg", bufs=3) as mp, \
             tc.tile_pool(name="ff", bufs=3) as fp_, \
             tc.tile_pool(name="small", bufs=4) as sp, \
             tc.tile_pool(name="oc", bufs=4) as ocp, \
             tc.tile_pool(name="pmp", bufs=3, space="PSUM") as pmp, \
             tc.tile_pool(name="obp", bufs=3, space="PSUM") as obp:
            wa_sb = cp.tile([H, H], dt.bfloat16)
            wb_sb = cp.tile([H, H], dt.bfloat16)
            wc_sb = cp.tile([INV + 1, H], dt.bfloat16)
            w2w_sb = cp.tile([128, NSUB, H], dt.bfloat16)
            b2t = cp.tile([128, 1], dt.float32)
            nc.vector.memset(b2t[:], b2val)
            nc.sync.dma_start(out=wa_sb[:], in_=wa[:, :])
            nc.sync.dma_start(out=wb_sb[:], in_=wb[:, :])
            nc.sync.dma_start(out=wc_sb[:], in_=wc[:, :])
            nc.sync.dma_start(out=w2w_sb[:], in_=w2w[:, :, :])

            for tp in range(nblocks // 2):
                ssp = slice(tp * 2 * BSLOT, (tp + 1) * 2 * BSLOT)
                xs_sb = stp.tile([128, 2 * BSLOT], dt.bfloat16, tag="xs")
                xr_sb = stp.tile([128, 2 * BSLOT], dt.bfloat16, tag="xr")
                se_sb = stp.tile([128, 2 * BSLOT], dt.bfloat16, tag="se")
                nc.sync.dma_start(out=xs_sb[:], in_=xst[:, ssp])
                nc.sync.dma_start(out=xr_sb[:], in_=xrt[:, ssp])
                nc.sync.dma_start(out=se_sb[:], in_=sel[:, ssp])
                ea_sb = eap.tile([INV + 1, 2 * BSLOT], dt.bfloat16, tag="ea")
                nc.sync.dma_start(out=ea_sb[:], in_=eat[:, ssp])
                oc = ocp.tile([128, 2, 128], dt.float16, tag="oc")
                for half in range(2):
                  t = 2 * tp + half
                  hof = half * BSLOT
                  pm = pmp.tile([128, NSUB, 128], dt.float32, tag="pm")
                  for j in range(NSUB):
                    js = slice(hof + j * 128, hof + (j + 1) * 128)
                    nc.tensor.matmul(out=pm[:, j, :], lhsT=xs_sb[:, js],
                                     rhs=wa_sb[:], start=True, stop=False)
                    nc.tensor.matmul(out=pm[:, j, :], lhsT=xr_sb[:, js],
                                     rhs=wb_sb[:], start=False, stop=False)
                    nc.tensor.matmul(out=pm[:, j, :], lhsT=ea_sb[:, js],
                                     rhs=wc_sb[:], start=False, stop=True)
                  sg_t = sgp.tile([128, NSUB, 128], dt.bfloat16, tag="sg")
                  nc.scalar.activation(
                    out=sg_t[:], in_=pm[:, :, :],
                    func=mybir.ActivationFunctionType.Sigmoid)
                  msg = mp.tile([128, NSUB, 128], dt.bfloat16, tag="msg")
                  nc.vector.tensor_tensor(out=msg[:], in0=pm[:, :, :],
                                        in1=sg_t[:], op=mybir.AluOpType.mult)
                  tt = mp.tile([128, NSUB, 128], dt.bfloat16, tag="tt")
                  nc.vector.tensor_tensor(out=tt[:], in0=msg[:], in1=w2w_sb[:],
                                        op=mybir.AluOpType.mult)
                  red = sp.tile([128, NSUB], dt.float32, tag="red")
                  nc.vector.tensor_reduce(out=red[:], in_=tt[:],
                                        axis=mybir.AxisListType.X,
                                        op=mybir.AluOpType.add)
                  gate = sp.tile([128, NSUB], dt.bfloat16, tag="gate")
                  nc.scalar.activation(
                    out=gate[:], in_=red[:],
                    func=mybir.ActivationFunctionType.Sigmoid,
                    bias=b2t[:, 0:1])
                  ffl = fp_.tile([128, NSUB, 128], dt.bfloat16, tag="ff")
                  nc.vector.tensor_tensor(
                    out=ffl[:], in0=msg[:],
                    in1=gate[:].to_broadcast([128, NSUB, 128]),
                    op=mybir.AluOpType.mult)
                  ob = obp.tile([128, 128], dt.float32, tag="ob")
                  for j in range(NSUB):
                    js = slice(hof + j * 128, hof + (j + 1) * 128)
                    nc.tensor.matmul(out=ob[:], lhsT=se_sb[:, js],
                                     rhs=ffl[:, j, :], start=(j == 0),
                                     stop=(j == NSUB - 1))
                  nc.vector.tensor_copy(out=oc[:, half, :], in_=ob[:])
                nc.sync.dma_start(
                    out=outp[tp * 256 : (tp + 1) * 256, :], in_=oc[:])
    nc.compile()
    _prog_cache[key] = nc
    return nc


def _host_prep(x_send, x_rec, index, edge_attr, bn_gamma, bn_beta, bn_mean,
               bn_var, W1, b1, W2, b2):
    s = np.asarray(index[0], dtype=np.int64)
    r = np.asarray(index[1], dtype=np.int64)
    ea = np.asarray(edge_attr, dtype=np.float32)

    scale = np.asarray(bn_gamma) / np.sqrt(np.asarray(bn_var) + BN_EPS)
    shift = np.asarray(bn_beta) - np.asarray(bn_mean) * scale
    W1f = (np.asarray(W1) * scale[:, None]).astype(np.float32)
    b1f = (np.asarray(b1) + shift @ np.asarray(W1)).astype(np.float32)

    wa = W1f[:H].astype(bfloat16)
    wb = W1f[H : 2 * H].astype(bfloat16)
    wc = np.concatenate([W1f[2 * H :], b1f[None, :]], axis=0).astype(bfloat16)
    w2w = np.broadcast_to(
        np.asarray(W2, dtype=np.float32).reshape(1, 1, H), (128, NSUB, H)
    ).astype(bfloat16).copy()
    b2val = float(np.asarray(b2).reshape(-1)[0])

    xs_b = np.asarray(x_send, dtype=np.float32).astype(bfloat16)
    xr_b = np.asarray(x_rec, dtype=np.float32).astype(bfloat16)
    core_of = (r // NLOC).astype(np.int64)

    # per-core receiver-block partition: <=128 consecutive receivers whose
    # edge count fits in BSLOT slots
    percore = []
    nblocks = 0
    for k in range(NCORES):
        m = core_of == k
        sk, rk, eak = s[m], (r[m] - k * NLOC), ea[m]
        o = np.argsort(rk, kind="stable")
        sk, rk, eak = sk[o], rk[o], eak[o]
        deg = np.bincount(rk, minlength=NLOC)
        cum = np.concatenate([[0], np.cumsum(deg)])
        bases = [0]
        while bases[-1] < NLOC:
            base = bases[-1]
            n = int(np.searchsorted(cum, cum[base] + BSLOT, side="right")) - 1
            n = min(n - base, 128)
            assert n >= 1, "receiver degree exceeds block capacity"
            bases.append(base + n)
        percore.append((sk, rk, eak, cum, np.asarray(bases)))
        nblocks = max(nblocks, len(bases) - 1)


    nblocks += nblocks % 2
    slots = nblocks * BSLOT
    in_maps = []
    unpack_meta = []
    for k in range(NCORES):
        sk, rk, eak, cum, bases = percore[k]
        blk_marks = np.zeros(NLOC + 1, dtype=np.int64)
        blk_marks[bases[:-1]] = 1
        blk_of_r = np.cumsum(blk_marks[: NLOC]) - 1
        t_e = blk_of_r[rk]                       # block of each edge
        slot = t_e * BSLOT + (np.arange(rk.size) - cum[bases[t_e]])
        xst = np.zeros((128, slots), dtype=bfloat16)
        xrts = np.zeros((128, slots), dtype=bfloat16)
        selm = np.zeros((128, slots), dtype=bfloat16)
        eat = np.zeros((INV + 1, slots), dtype=np.float32)
        xst[:, slot] = xs_b[sk].T
        xrts[:, slot] = xr_b[k * NLOC + rk].T
        eat[:INV, slot] = eak.T
        eat[INV, slot] = 1.0
        # Sel lhsT tile per subtile: [e-part, r_rel free]; edge at slot has
        # partition slot%128 and free column r_rel inside its subtile group
        r_rel = (rk - bases[t_e]).astype(np.int64)
        p = slot % 128
        selm[p, (slot // 128) * 128 + r_rel] = 1.0
        in_maps.append({
            "xst": xst, "xrt": xrts, "sel": selm,
            "eat": eat.astype(bfloat16),
            "wa": wa, "wb": wb, "wc": wc, "w2w": w2w,
        })
        unpack_meta.append(bases)
    return in_maps, b2val, nblocks, unpack_meta


def kernel(**inputs) -> np.ndarray:
    in_maps, b2val, nblocks, unpack_meta = _host_prep(**inputs)
    nc = _build(b2val, nblocks)
    res = run_bass_kernel_spmd(nc, in_maps, core_ids=list(range(NCORES)))
    out = np.zeros((N, H), dtype=np.float32)
    for k in range(NCORES):
        op = np.asarray(res.results[k]["outp"], dtype=np.float32)
        # pair-batched writeback: block t row p lives at
        # op[(t//2)*256 + 2*p + t%2]
        opv = op.reshape(-1, 128, 2, H)
        bases = unpack_meta[k]
        for t in range(len(bases) - 1):
            nrec = bases[t + 1] - bases[t]
            out[k * NLOC + bases[t] : k * NLOC + bases[t + 1]] = \
                opv[t // 2, :nrec, t % 2, :]
    return out


# revision 18
# speedup vs baseline: 63679.5409x; 1.2102x over previous
"""ETNN messager layer on 8 Trainium2 NeuronCores — v5 (streaming, no indirect DMA).

Receiver-sharded: core k owns receivers [k*12500,(k+1)*12500). The host
stages per-edge data as contiguous streams (pure data movement: gather /
transpose / one-hot layout); the device does all model FLOPs and sees only
contiguous DMA:

- xsT / xrT: per-edge x_send / x_rec rows, transposed to [H, slots] bf16,
  consumed directly as matmul lhsT tiles (z = xs@Wa + xr@Wb + ea@Wc + b1,
  BN folded into W1 on host).
- Edges are grouped into fixed-size receiver blocks: each block = up to 128
  consecutive receivers whose edges fit in 512 slots (zero-padded). The
  segment-sum is a one-hot matmul per 128-slot subtile: out_block[128,H]
  += Sel^T @ ff accumulated in PSUM (fp32), then written contiguously to
  the output table. Sel columns of pad slots are all-zero, so pads vanish.
- msg = z*sigmoid(z); gate = sigmoid(msg.W2+b2); ff = msg*gate.

The block structure is data-dependent but the program is uniform (every
block = 4 subtiles), so one SPMD program serves all 8 cores; the host
compacts each core's [nblocks*128, H] output using its block bases.
"""

import numpy as np
from ml_dtypes import bfloat16

import concourse.tile as tile
from concourse import bacc, bass, mybir
from concourse.bass_utils import run_bass_kernel_spmd

N = 100000
E = 500000
H = 128
INV = 16
NCORES = 8
NLOC = N // NCORES           # 12500 receivers per core
BSLOT = 512                  # slots per receiver block (4 subtiles)
NSUB = BSLOT // 128
BN_EPS = 1e-5

_prog_cache = {}


def _build(b2val: float, nblocks: int):
    key = (round(b2val, 9), nblocks)
    if key in _prog_cache:
        return _prog_cache[key]
    slots = nblocks * BSLOT

    nc = bacc.Bacc("TRN2", target_bir_lowering=False, debug=False)
    dt = mybir.dt
    xst = nc.dram_tensor("xst", [128, slots], dt.bfloat16, kind="ExternalInput")
    xrt = nc.dram_tensor("xrt", [128, slots], dt.bfloat16, kind="ExternalInput")
    sel = nc.dram_tensor("sel", [128, slots], dt.bfloat16, kind="ExternalInput")
    eat = nc.dram_tensor("eat", [INV + 1, slots], dt.bfloat16,
                         kind="ExternalInput")
    wa = nc.dram_tensor("wa", [H, H], dt.bfloat16, kind="ExternalInput")
    wb = nc.dram_tensor("wb", [H, H], dt.bfloat16, kind="ExternalInput")
    wc = nc.dram_tensor("wc", [INV + 1, H], dt.bfloat16, kind="ExternalInput")
    w2w = nc.dram_tensor("w2w", [128, NSUB, H], dt.bfloat16,
                         kind="ExternalInput")
    outp = nc.dram_tensor("outp", [nblocks * 128, H], dt.float16,
                          kind="ExternalOutput")

    with tile.TileContext(nc) as tc:
        with tc.tile_pool(name="const", bufs=1) as cp, \
             tc.tile_pool(name="strm", bufs=3) as stp, \
             tc.tile_pool(name="ea", bufs=3) as eap, \
             tc.tile_pool(name="sgp", bufs=3) as sgp, \
             tc.tile_pool(name="msg", bufs=3) as mp, \
             tc.tile_pool(name="ff", bufs=3) as fp_, \
             tc.tile_pool(name="small", bufs=4) as sp, \
             tc.tile_pool(name="oc", bufs=4) as ocp, \
             tc.tile_pool(name="pmp", bufs=3, space="PSUM") as pmp, \
             tc.tile_pool(name="obp", bufs=3, space="PSUM") as obp:
            wa_sb = cp.tile([H, H], dt.bfloat16)
            wb_sb = cp.tile([H, H], dt.bfloat16)
            wc_sb = cp.tile([INV + 1, H], dt.bfloat16)
            w2w_sb = cp.tile([128, NSUB, H], dt.bfloat16)
            b2t = cp.tile([128, 1], dt.float32)
            nc.vector.memset(b2t[:], b2val)
            nc.sync.dma_start(out=wa_sb[:], in_=wa[:, :])
            nc.sync.dma_start(out=wb_sb[:], in_=wb[:, :])
            nc.sync.dma_start(out=wc_sb[:], in_=wc[:, :])
            nc.sync.dma_start(out=w2w_sb[:], in_=w2w[:, :, :])

            for tp in range(nblocks // 2):
                ssp = slice(tp * 2 * BSLOT, (tp + 1) * 2 * BSLOT)
                xs_sb = stp.tile([128, 2 * BSLOT], dt.bfloat16, tag="xs")
                xr_sb = stp.tile([128, 2 * BSLOT], dt.bfloat16, tag="xr")
                se_sb = stp.tile([128, 2 * BSLOT], dt.bfloat16, tag="se")
                nc.sync.dma_start(out=xs_sb[:], in_=xst[:, ssp])
                nc.sync.dma_start(out=xr_sb[:], in_=xrt[:, ssp])
                nc.sync.dma_start(out=se_sb[:], in_=sel[:, ssp])
                ea_sb = eap.tile([INV + 1, 2 * BSLOT], dt.bfloat16, tag="ea")
                nc.sync.dma_start(out=ea_sb[:], in_=eat[:, ssp])
                oc = ocp.tile([128, 2, 128], dt.float16, tag="oc")
                for half in range(2):
                  t = 2 * tp + half
                  hof = half * BSLOT
                  pm = pmp.tile([128, NSUB, 128], dt.float32, tag="pm")
                  for j in range(NSUB):
                    js = slice(hof + j * 128, hof + (j + 1) * 128)
                    nc.tensor.matmul(out=pm[:, j, :], lhsT=xs_sb[:, js],
                                     rhs=wa_sb[:], start=True, stop=False)
                    nc.tensor.matmul(out=pm[:, j, :], lhsT=xr_sb[:, js],
                                     rhs=wb_sb[:], start=False, stop=False)
                    nc.tensor.matmul(out=pm[:, j, :], lhsT=ea_sb[:, js],
                                     rhs=wc_sb[:], start=False, stop=True)
                  msg = mp.tile([128, NSUB, 128], dt.bfloat16, tag="msg")
                  nc.scalar.activation(
                    out=msg[:], in_=pm[:, :, :],
                    func=mybir.ActivationFunctionType.Silu)
                  tt = mp.tile([128, NSUB, 128], dt.bfloat16, tag="tt")
                  nc.vector.tensor_tensor(out=tt[:], in0=msg[:], in1=w2w_sb[:],
                                        op=mybir.AluOpType.mult)
                  red = sp.tile([128, NSUB], dt.float32, tag="red")
                  nc.vector.tensor_reduce(out=red[:], in_=tt[:],
                                        axis=mybir.AxisListType.X,
                                        op=mybir.AluOpType.add)
                  gate = sp.tile([128, NSUB], dt.bfloat16, tag="gate")
                  nc.scalar.activation(
                    out=gate[:], in_=red[:],
                    func=mybir.ActivationFunctionType.Sigmoid,
                    bias=b2t[:, 0:1])
                  ffl = fp_.tile([128, NSUB, 128], dt.bfloat16, tag="ff")
                  nc.vector.tensor_tensor(
                    out=ffl[:], in0=msg[:],
                    in1=gate[:].to_broadcast([128, NSUB, 128]),
                    op=mybir.AluOpType.mult)
                  ob = obp.tile([128, 128], dt.float32, tag="ob")
                  for j in range(NSUB):
                    js = slice(hof + j * 128, hof + (j + 1) * 128)
                    nc.tensor.matmul(out=ob[:], lhsT=se_sb[:, js],
                                     rhs=ffl[:, j, :], start=(j == 0),
                                     stop=(j == NSUB - 1))
                  nc.vector.tensor_copy(out=oc[:, half, :], in_=ob[:])
                nc.sync.dma_start(
                    out=outp[tp * 256 : (tp + 1) * 256, :], in_=oc[:])
    nc.compile()
    _prog_cache[key] = nc
    return nc


def _host_prep(x_send, x_rec, index, edge_attr, bn_gamma, bn_beta, bn_mean,
               bn_var, W1, b1, W2, b2):
    s = np.asarray(index[0], dtype=np.int64)
    r = np.asarray(index[1], dtype=np.int64)
    ea = np.asarray(edge_attr, dtype=np.float32)

    scale = np.asarray(bn_gamma) / np.sqrt(np.asarray(bn_var) + BN_EPS)
    shift = np.asarray(bn_beta) - np.asarray(bn_mean) * scale
    W1f = (np.asarray(W1) * scale[:, None]).astype(np.float32)
    b1f = (np.asarray(b1) + shift @ np.asarray(W1)).astype(np.float32)

    wa = W1f[:H].astype(bfloat16)
    wb = W1f[H : 2 * H].astype(bfloat16)
    wc = np.concatenate([W1f[2 * H :], b1f[None, :]], axis=0).astype(bfloat16)
    w2w = np.broadcast_to(
        np.asarray(W2, dtype=np.float32).reshape(1, 1, H), (128, NSUB, H)
    ).astype(bfloat16).copy()
    b2val = float(np.asarray(b2).reshape(-1)[0])

    xs_b = np.asarray(x_send, dtype=np.float32).astype(bfloat16)
    xr_b = np.asarray(x_rec, dtype=np.float32).astype(bfloat16)
    core_of = (r // NLOC).astype(np.int64)

    # per-core receiver-block partition: <=128 consecutive receivers whose
    # edge count fits in BSLOT slots
    percore = []
    nblocks = 0
    for k in range(NCORES):
        m = core_of == k
        sk, rk, eak = s[m], (r[m] - k * NLOC), ea[m]
        o = np.argsort(rk, kind="stable")
        sk, rk, eak = sk[o], rk[o], eak[o]
        deg = np.bincount(rk, minlength=NLOC)
        cum = np.concatenate([[0], np.cumsum(deg)])
        bases = [0]
        while bases[-1] < NLOC:
            base = bases[-1]
            n = int(np.searchsorted(cum, cum[base] + BSLOT, side="right")) - 1
            n = min(n - base, 128)
            assert n >= 1, "receiver degree exceeds block capacity"
            bases.append(base + n)
        percore.append((sk, rk, eak, cum, np.asarray(bases)))
        nblocks = max(nblocks, len(bases) - 1)


    nblocks += nblocks % 2
    slots = nblocks * BSLOT
    in_maps = []
    unpack_meta = []
    for k in range(NCORES):
        sk, rk, eak, cum, bases = percore[k]
        blk_marks = np.zeros(NLOC + 1, dtype=np.int64)
        blk_marks[bases[:-1]] = 1
        blk_of_r = np.cumsum(blk_marks[: NLOC]) - 1
        t_e = blk_of_r[rk]                       # block of each edge
        slot = t_e * BSLOT + (np.arange(rk.size) - cum[bases[t_e]])
        xst = np.zeros((128, slots), dtype=bfloat16)
        xrts = np.zeros((128, slots), dtype=bfloat16)
        selm = np.zeros((128, slots), dtype=bfloat16)
        eat = np.zeros((INV + 1, slots), dtype=np.float32)
        xst[:, slot] = xs_b[sk].T
        xrts[:, slot] = xr_b[k * NLOC + rk].T
        eat[:INV, slot] = eak.T
        eat[INV, slot] = 1.0
        # Sel lhsT tile per subtile: [e-part, r_rel free]; edge at slot has
        # partition slot%128 and free column r_rel inside its subtile group
        r_rel = (rk - bases[t_e]).astype(np.int64)
        p = slot % 128
        selm[p, (slot // 128) * 128 + r_rel] = 1.0
        in_maps.append({
            "xst": xst, "xrt": xrts, "sel": selm,
            "eat": eat.astype(bfloat16),
            "wa": wa, "wb": wb, "wc": wc, "w2w": w2w,
        })
        unpack_meta.append(bases)
    return in_maps, b2val, nblocks, unpack_meta


def kernel(**inputs) -> np.ndarray:
    in_maps, b2val, nblocks, unpack_meta = _host_prep(**inputs)
    nc = _build(b2val, nblocks)
    res = run_bass_kernel_spmd(nc, in_maps, core_ids=list(range(NCORES)))
    out = np.zeros((N, H), dtype=np.float32)
    for k in range(NCORES):
        op = np.asarray(res.results[k]["outp"], dtype=np.float32)
        # pair-batched writeback: block t row p lives at
        # op[(t//2)*256 + 2*p + t%2]
        opv = op.reshape(-1, 128, 2, H)
        bases = unpack_meta[k]
        for t in range(len(bases) - 1):
            nrec = bases[t + 1] - bases[t]
            out[k * NLOC + bases[t] : k * NLOC + bases[t + 1]] = \
                opv[t // 2, :nrec, t % 2, :]
    return out


# revision 19
# speedup vs baseline: 64967.6804x; 1.0202x over previous
"""ETNN messager layer on 8 Trainium2 NeuronCores — v5 (streaming, no indirect DMA).

Receiver-sharded: core k owns receivers [k*12500,(k+1)*12500). The host
stages per-edge data as contiguous streams (pure data movement: gather /
transpose / one-hot layout); the device does all model FLOPs and sees only
contiguous DMA:

- xsT / xrT: per-edge x_send / x_rec rows, transposed to [H, slots] bf16,
  consumed directly as matmul lhsT tiles (z = xs@Wa + xr@Wb + ea@Wc + b1,
  BN folded into W1 on host).
- Edges are grouped into fixed-size receiver blocks: each block = up to 128
  consecutive receivers whose edges fit in 512 slots (zero-padded). The
  segment-sum is a one-hot matmul per 128-slot subtile: out_block[128,H]
  += Sel^T @ ff accumulated in PSUM (fp32), then written contiguously to
  the output table. Sel columns of pad slots are all-zero, so pads vanish.
- msg = z*sigmoid(z); gate = sigmoid(msg.W2+b2); ff = msg*gate.

The block structure is data-dependent but the program is uniform (every
block = 4 subtiles), so one SPMD program serves all 8 cores; the host
compacts each core's [nblocks*128, H] output using its block bases.
"""

import numpy as np
from ml_dtypes import bfloat16

import concourse.tile as tile
from concourse import bacc, bass, mybir
from concourse.bass_utils import run_bass_kernel_spmd

N = 100000
E = 500000
H = 128
INV = 16
NCORES = 8
NLOC = N // NCORES           # 12500 receivers per core
BSLOT = 512                  # slots per receiver block (4 subtiles)
NSUB = BSLOT // 128
BN_EPS = 1e-5

_prog_cache = {}


def _build(b2val: float, nblocks: int):
    key = (round(b2val, 9), nblocks)
    if key in _prog_cache:
        return _prog_cache[key]
    slots = nblocks * BSLOT

    nc = bacc.Bacc("TRN2", target_bir_lowering=False, debug=False)
    dt = mybir.dt
    xst = nc.dram_tensor("xst", [128, slots], dt.bfloat16, kind="ExternalInput")
    xrt = nc.dram_tensor("xrt", [128, slots], dt.bfloat16, kind="ExternalInput")
    sel = nc.dram_tensor("sel", [128, slots], dt.bfloat16, kind="ExternalInput")
    eat = nc.dram_tensor("eat", [INV + 1, slots], dt.bfloat16,
                         kind="ExternalInput")
    wa = nc.dram_tensor("wa", [H, H], dt.bfloat16, kind="ExternalInput")
    wb = nc.dram_tensor("wb", [H, H], dt.bfloat16, kind="ExternalInput")
    wc = nc.dram_tensor("wc", [INV + 1, H], dt.bfloat16, kind="ExternalInput")
    w2w = nc.dram_tensor("w2w", [128, 4 * NSUB, H], dt.bfloat16,
                         kind="ExternalInput")
    outp = nc.dram_tensor("outp", [nblocks * 128, H], dt.float16,
                          kind="ExternalOutput")

    with tile.TileContext(nc) as tc:
        with tc.tile_pool(name="const", bufs=1) as cp, \
             tc.tile_pool(name="strm", bufs=3) as stp, \
             tc.tile_pool(name="ea", bufs=3) as eap, \
             tc.tile_pool(name="sgp", bufs=3) as sgp, \
             tc.tile_pool(name="msg", bufs=3) as mp, \
             tc.tile_pool(name="ff", bufs=3) as fp_, \
             tc.tile_pool(name="small", bufs=4) as sp, \
             tc.tile_pool(name="oc", bufs=4) as ocp, \
             tc.tile_pool(name="pmp", bufs=4, space="PSUM") as pmp, \
             tc.tile_pool(name="obp", bufs=4, space="PSUM") as obp:
            wa_sb = cp.tile([H, H], dt.bfloat16)
            wb_sb = cp.tile([H, H], dt.bfloat16)
            wc_sb = cp.tile([INV + 1, H], dt.bfloat16)
            w2w_sb = cp.tile([128, 4 * NSUB, H], dt.bfloat16)
            b2t = cp.tile([128, 1], dt.float32)
            nc.vector.memset(b2t[:], b2val)
            nc.sync.dma_start(out=wa_sb[:], in_=wa[:, :])
            nc.sync.dma_start(out=wb_sb[:], in_=wb[:, :])
            nc.sync.dma_start(out=wc_sb[:], in_=wc[:, :])
            nc.sync.dma_start(out=w2w_sb[:], in_=w2w[:, :, :])

            NQ = 4 * NSUB       # subtiles per quad
            for tq in range(nblocks // 4):
                ssq = slice(tq * 4 * BSLOT, (tq + 1) * 4 * BSLOT)
                xs_sb = stp.tile([128, 4 * BSLOT], dt.bfloat16, tag="xs")
                xr_sb = stp.tile([128, 4 * BSLOT], dt.bfloat16, tag="xr")
                se_sb = stp.tile([128, 4 * BSLOT], dt.bfloat16, tag="se")
                nc.sync.dma_start(out=xs_sb[:], in_=xst[:, ssq])
                nc.sync.dma_start(out=xr_sb[:], in_=xrt[:, ssq])
                nc.sync.dma_start(out=se_sb[:], in_=sel[:, ssq])
                ea_sb = eap.tile([INV + 1, 4 * BSLOT], dt.bfloat16, tag="ea")
                nc.sync.dma_start(out=ea_sb[:], in_=eat[:, ssq])
                oc = ocp.tile([128, 4, 128], dt.float16, tag="oc")
                msg = mp.tile([128, NQ, 128], dt.bfloat16, tag="msg")
                for q in range(4):
                  hof = q * BSLOT
                  pm = pmp.tile([128, NSUB, 128], dt.float32, tag="pm")
                  for j in range(NSUB):
                    js = slice(hof + j * 128, hof + (j + 1) * 128)
                    nc.tensor.matmul(out=pm[:, j, :], lhsT=xs_sb[:, js],
                                     rhs=wa_sb[:], start=True, stop=False)
                    nc.tensor.matmul(out=pm[:, j, :], lhsT=xr_sb[:, js],
                                     rhs=wb_sb[:], start=False, stop=False)
                    nc.tensor.matmul(out=pm[:, j, :], lhsT=ea_sb[:, js],
                                     rhs=wc_sb[:], start=False, stop=True)
                  nc.scalar.activation(
                    out=msg[:, q * NSUB : (q + 1) * NSUB, :], in_=pm[:, :, :],
                    func=mybir.ActivationFunctionType.Silu)
                tt = mp.tile([128, NQ, 128], dt.bfloat16, tag="tt")
                nc.vector.tensor_tensor(out=tt[:], in0=msg[:], in1=w2w_sb[:],
                                        op=mybir.AluOpType.mult)
                red = sp.tile([128, NQ], dt.float32, tag="red")
                nc.vector.tensor_reduce(out=red[:], in_=tt[:],
                                        axis=mybir.AxisListType.X,
                                        op=mybir.AluOpType.add)
                gate = sp.tile([128, NQ], dt.bfloat16, tag="gate")
                nc.scalar.activation(
                    out=gate[:], in_=red[:],
                    func=mybir.ActivationFunctionType.Sigmoid,
                    bias=b2t[:, 0:1])
                ffl = fp_.tile([128, NQ, 128], dt.bfloat16, tag="ff")
                nc.vector.tensor_tensor(
                    out=ffl[:], in0=msg[:],
                    in1=gate[:].to_broadcast([128, NQ, 128]),
                    op=mybir.AluOpType.mult)
                for q in range(4):
                  hof = q * BSLOT
                  ob = obp.tile([128, 128], dt.float32, tag="ob")
                  for j in range(NSUB):
                    js = slice(hof + j * 128, hof + (j + 1) * 128)
                    nc.tensor.matmul(out=ob[:], lhsT=se_sb[:, js],
                                     rhs=ffl[:, q * NSUB + j, :],
                                     start=(j == 0), stop=(j == NSUB - 1))
                  nc.vector.tensor_copy(out=oc[:, q, :], in_=ob[:])
                nc.sync.dma_start(
                    out=outp[tq * 512 : (tq + 1) * 512, :], in_=oc[:])
    nc.compile()
    _prog_cache[key] = nc
    return nc


def _host_prep(x_send, x_rec, index, edge_attr, bn_gamma, bn_beta, bn_mean,
               bn_var, W1, b1, W2, b2):
    s = np.asarray(index[0], dtype=np.int64)
    r = np.asarray(index[1], dtype=np.int64)
    ea = np.asarray(edge_attr, dtype=np.float32)

    scale = np.asarray(bn_gamma) / np.sqrt(np.asarray(bn_var) + BN_EPS)
    shift = np.asarray(bn_beta) - np.asarray(bn_mean) * scale
    W1f = (np.asarray(W1) * scale[:, None]).astype(np.float32)
    b1f = (np.asarray(b1) + shift @ np.asarray(W1)).astype(np.float32)

    wa = W1f[:H].astype(bfloat16)
    wb = W1f[H : 2 * H].astype(bfloat16)
    wc = np.concatenate([W1f[2 * H :], b1f[None, :]], axis=0).astype(bfloat16)
    w2w = np.broadcast_to(
        np.asarray(W2, dtype=np.float32).reshape(1, 1, H), (128, 4 * NSUB, H)
    ).astype(bfloat16).copy()
    b2val = float(np.asarray(b2).reshape(-1)[0])

    xs_b = np.asarray(x_send, dtype=np.float32).astype(bfloat16)
    xr_b = np.asarray(x_rec, dtype=np.float32).astype(bfloat16)
    core_of = (r // NLOC).astype(np.int64)

    # per-core receiver-block partition: <=128 consecutive receivers whose
    # edge count fits in BSLOT slots
    percore = []
    nblocks = 0
    for k in range(NCORES):
        m = core_of == k
        sk, rk, eak = s[m], (r[m] - k * NLOC), ea[m]
        o = np.argsort(rk, kind="stable")
        sk, rk, eak = sk[o], rk[o], eak[o]
        deg = np.bincount(rk, minlength=NLOC)
        cum = np.concatenate([[0], np.cumsum(deg)])
        bases = [0]
        while bases[-1] < NLOC:
            base = bases[-1]
            n = int(np.searchsorted(cum, cum[base] + BSLOT, side="right")) - 1
            n = min(n - base, 128)
            assert n >= 1, "receiver degree exceeds block capacity"
            bases.append(base + n)
        percore.append((sk, rk, eak, cum, np.asarray(bases)))
        nblocks = max(nblocks, len(bases) - 1)


    nblocks += (-nblocks) % 4
    slots = nblocks * BSLOT
    in_maps = []
    unpack_meta = []
    for k in range(NCORES):
        sk, rk, eak, cum, bases = percore[k]
        blk_marks = np.zeros(NLOC + 1, dtype=np.int64)
        blk_marks[bases[:-1]] = 1
        blk_of_r = np.cumsum(blk_marks[: NLOC]) - 1
        t_e = blk_of_r[rk]                       # block of each edge
        slot = t_e * BSLOT + (np.arange(rk.size) - cum[bases[t_e]])
        xst = np.zeros((128, slots), dtype=bfloat16)
        xrts = np.zeros((128, slots), dtype=bfloat16)
        selm = np.zeros((128, slots), dtype=bfloat16)
        eat = np.zeros((INV + 1, slots), dtype=np.float32)
        xst[:, slot] = xs_b[sk].T
        xrts[:, slot] = xr_b[k * NLOC + rk].T
        eat[:INV, slot] = eak.T
        eat[INV, slot] = 1.0
        # Sel lhsT tile per subtile: [e-part, r_rel free]; edge at slot has
        # partition slot%128 and free column r_rel inside its subtile group
        r_rel = (rk - bases[t_e]).astype(np.int64)
        p = slot % 128
        selm[p, (slot // 128) * 128 + r_rel] = 1.0
        in_maps.append({
            "xst": xst, "xrt": xrts, "sel": selm,
            "eat": eat.astype(bfloat16),
            "wa": wa, "wb": wb, "wc": wc, "w2w": w2w,
        })
        unpack_meta.append(bases)
    return in_maps, b2val, nblocks, unpack_meta


def kernel(**inputs) -> np.ndarray:
    in_maps, b2val, nblocks, unpack_meta = _host_prep(**inputs)
    nc = _build(b2val, nblocks)
    res = run_bass_kernel_spmd(nc, in_maps, core_ids=list(range(NCORES)))
    out = np.zeros((N, H), dtype=np.float32)
    for k in range(NCORES):
        op = np.asarray(res.results[k]["outp"], dtype=np.float32)
        # pair-batched writeback: block t row p lives at
        # op[(t//2)*256 + 2*p + t%2]
        opv = op.reshape(-1, 128, 4, H)
        bases = unpack_meta[k]
        for t in range(len(bases) - 1):
            nrec = bases[t + 1] - bases[t]
            out[k * NLOC + bases[t] : k * NLOC + bases[t + 1]] = \
                opv[t // 4, :nrec, t % 4, :]
    return out
